# revision 2
# baseline (speedup 1.0000x reference)
"""BoundaryLoss Trainium2 kernel.

loss = mean(softmax(x, axis=1) * bdistmap) over [B,C,H,W], where bdistmap is
built from exact 2D Euclidean distance transforms of the per-class pos/neg
masks (reference uses separable min-plus EDT with BIG=1e9 standing in for inf).

Strategy (data-parallel over batch, one image per NeuronCore, 8 cores):
  per (b, c, mask-type):
    pass 1  - exact 1D nearest-True distance along H via two sequential
              min-plus scans (DVE TensorTensorScan: state=min(state+1, g)),
              computed in transposed layout (H on the free axis).
    square+clamp -> g1 = min(dist^2, 1e9), PE-transpose back to natural layout.
    pass 2  - parabolic min-plus along W, offsets |k| <= K:
              d2 = min_k (k^2 + g1 shifted by k). K is derived on the host
              from the labels (sound upper bound on the optimal offset), so
              the device computation is exact.
  bdistmap = sqrt(d2_pos) - sqrt(d2_neg)   (identical to the masked form since
              EDT(mask)=0 on mask pixels and pos/neg masks are complements)
  partial = sum over image of softmax * bdistmap  -> [128,1] per core,
  host sums the 8 partials and divides by B*C*H*W.
"""
import os
import numpy as np

import concourse.bass as bass
import concourse.tile as tile
from concourse import bacc, mybir
from concourse.masks import make_identity
from concourse.bass_utils import run_bass_kernel_spmd

F32 = mybir.dt.float32
I32 = mybir.dt.int32
AF = mybir.ActivationFunctionType
OP = mybir.AluOpType

B, C, H, W = 8, 4, 256, 256
INF = 1.0e9

LAST_RESULT = None
_BUILD_CACHE = {}


def _emit(tc, x_d, y_d, out_d, K):
    nc = tc.nc
    PAD = max(K, 1)
    WB = W + 2 * PAD

    from contextlib import ExitStack
    ctx = ExitStack()
    pool = ctx.enter_context(tc.tile_pool(name="main", bufs=1))
    psum = ctx.enter_context(tc.tile_pool(name="psum", bufs=4, space="PSUM"))

    ones = pool.tile([128, H], F32)
    nc.vector.memset(ones[:], 1.0)
    ident = pool.tile([128, 128], F32)
    make_identity(nc, ident[:])

    # load labels, int -> f32 (values in [0,4): exact)
    y_sb = pool.tile([128, 2, W], I32)
    for ha in range(2):
        nc.sync.dma_start(out=y_sb[:, ha, :], in_=y_d[0, ha * 128:(ha + 1) * 128, :])
    yf = pool.tile([128, 2, W], F32)
    nc.scalar.copy(yf[:], y_sb[:])

    # transpose labels to layout B (w on partitions, h on free axis)
    yT = pool.tile([128, 2, H], F32)
    for ha in range(2):
        for wb in range(2):
            pt = psum.tile([128, 128], F32)
            nc.tensor.transpose(pt[:], yf[:, ha, wb * 128:(wb + 1) * 128], ident[:])
            nc.scalar.copy(yT[:, wb, ha * 128:(ha + 1) * 128], pt[:])

    # scan init (pos masks only): 0 where y==c, INF elsewhere.
    # (neg-mask EDTs are derived later: the posmasks partition the image, so
    #  d2_neg_c = min_{c'!=c} d2_pos_c'.)
    init = pool.tile([128, C, 2, H], F32)
    for c in range(C):
        nc.vector.tensor_scalar(
            init[:, c, :, :].rearrange("p a h -> p (a h)"),
            yT[:].rearrange("p a h -> p (a h)"), float(c), INF,
            OP.not_equal, OP.mult)

    # pass 1: exact 1D distance along H via fwd+bwd min-plus scans
    fw = pool.tile([128, C, 2, H], F32)
    dw = pool.tile([128, C, 2, H], F32)
    for c in range(C):
        for wb in range(2):
            nc.vector.tensor_tensor_scan(
                fw[:, c, wb, :], ones[:], init[:, c, wb, :], INF,
                OP.add, OP.min)
            nc.vector.tensor_tensor_scan(
                dw[:, c, wb, ::-1], ones[:], fw[:, c, wb, ::-1], INF,
                OP.add, OP.min)

    # g1 = min(dist^2, INF)  (clamp matches reference's BIG fallback exactly)
    g1b = pool.tile([128, C, 2, H], F32)
    nc.scalar.activation(g1b[:], dw[:], AF.Square)
    nc.vector.tensor_scalar_min(g1b[:], g1b[:], INF)

    # transpose g1 to natural layout with INF pads of width PAD on both sides
    g1a = pool.tile([128, C, 2, WB], F32)
    flat = g1a[:].rearrange("p c h x -> p (c h) x")
    nc.gpsimd.memset(flat[:, :, 0:PAD], INF)
    nc.gpsimd.memset(flat[:, :, PAD + W:], INF)
    for c in range(C):
        for ha in range(2):
            for wb in range(2):
                pt = psum.tile([128, 128], F32)
                nc.tensor.transpose(
                    pt[:], g1b[:, c, wb, ha * 128:(ha + 1) * 128], ident[:])
                nc.scalar.copy(
                    g1a[:, c, ha, PAD + wb * 128: PAD + (wb + 1) * 128], pt[:])

    # pass 2: d2 = min_{|k|<=K} (k^2 + g1 shifted by k) along W
    acc = pool.tile([128, C, 2, W], F32)
    ctr = g1a[:, :, :, PAD:PAD + W]
    if K == 0:
        nc.vector.tensor_copy(acc[:], ctr)
    for k in range(1, K + 1):
        prev = ctr if k == 1 else acc[:]
        nc.vector.scalar_tensor_tensor(
            acc[:], g1a[:, :, :, PAD + k:PAD + k + W], float(k * k), prev,
            OP.add, OP.min)
        nc.vector.scalar_tensor_tensor(
            acc[:], g1a[:, :, :, PAD - k:PAD - k + W], float(k * k), acc[:],
            OP.add, OP.min)

    # d2_neg_c = min_{c'!=c} d2_pos_c'
    m01 = pool.tile([128, 2, W], F32)
    m23 = pool.tile([128, 2, W], F32)
    nc.vector.tensor_tensor(m01[:], acc[:, 0], acc[:, 1], OP.min)
    nc.vector.tensor_tensor(m23[:], acc[:, 2], acc[:, 3], OP.min)
    negd2 = pool.tile([128, C, 2, W], F32)
    nc.vector.tensor_tensor(negd2[:, 0], acc[:, 1], m23[:], OP.min)
    nc.vector.tensor_tensor(negd2[:, 1], acc[:, 0], m23[:], OP.min)
    nc.vector.tensor_tensor(negd2[:, 2], m01[:], acc[:, 3], OP.min)
    nc.vector.tensor_tensor(negd2[:, 3], m01[:], acc[:, 2], OP.min)

    # bdistmap = sqrt(d2_pos) - sqrt(d2_neg)
    dpos = pool.tile([128, C, 2, W], F32)
    dneg = pool.tile([128, C, 2, W], F32)
    nc.scalar.activation(dpos[:], acc[:], AF.Sqrt)
    nc.scalar.activation(dneg[:], negd2[:], AF.Sqrt)
    bd = pool.tile([128, C, 2, W], F32)
    nc.vector.tensor_sub(bd[:], dpos[:], dneg[:])

    # softmax-weighted partial sum (softmax without max-subtraction: |x|~N(0,1))
    x_sb = pool.tile([128, C, 2, W], F32)
    for c in range(C):
        for ha in range(2):
            nc.sync.dma_start(out=x_sb[:, c, ha, :],
                              in_=x_d[c, ha * 128:(ha + 1) * 128, :])
    ex = pool.tile([128, C, 2, W], F32)
    nc.scalar.activation(ex[:], x_sb[:], AF.Exp)
    den = pool.tile([128, 2, W], F32)
    nc.vector.tensor_add(den[:], ex[:, 0], ex[:, 1])
    nc.vector.tensor_add(den[:], den[:], ex[:, 2])
    nc.vector.tensor_add(den[:], den[:], ex[:, 3])
    rec = pool.tile([128, 2, W], F32)
    nc.vector.reciprocal(rec[:], den[:])
    num = pool.tile([128, 2, W], F32)
    nc.vector.tensor_mul(num[:], ex[:, 0], bd[:, 0])
    for c in range(1, C):
        tmp = pool.tile([128, 2, W], F32, tag="numtmp")
        nc.vector.tensor_mul(tmp[:], ex[:, c], bd[:, c])
        nc.vector.tensor_add(num[:], num[:], tmp[:])
    # (tensor_tensor_reduce crashes the device on this runtime; mul+reduce)
    ratio = pool.tile([128, 2, W], F32)
    part = pool.tile([128, 1], F32)
    nc.vector.tensor_mul(ratio[:], num[:], rec[:])
    nc.vector.tensor_reduce(part[:], ratio[:].rearrange("p a w -> p (a w)"),
                            op=OP.add, axis=mybir.AxisListType.X)
    nc.sync.dma_start(out=out_d[:], in_=part[:])
    ctx.close()


def _build(K):
    if K in _BUILD_CACHE:
        return _BUILD_CACHE[K]
    nc = bacc.Bacc("TRN2", target_bir_lowering=False)
    x_d = nc.dram_tensor("x", [C, H, W], F32, kind="ExternalInput")
    y_d = nc.dram_tensor("y_", [1, H, W], I32, kind="ExternalInput")
    out_d = nc.dram_tensor("out", [128, 1], F32, kind="ExternalOutput")
    with tile.TileContext(nc) as tc:
        _emit(tc, x_d, y_d, out_d, K)
    nc.compile()
    _BUILD_CACHE[K] = nc
    return nc


def _dist1d(mask, axis):
    """Exact 1D nearest-True distance along `axis` (doubling min-plus scans)."""
    m = np.moveaxis(mask, axis, -1)
    a = np.where(m, 0.0, INF).astype(np.float32)
    s = 1
    while s < m.shape[-1]:
        a[..., s:] = np.minimum(a[..., s:], a[..., :-s] + s)
        a[..., :-s] = np.minimum(a[..., :-s], a[..., s:] + s)
        s *= 2
    return np.moveaxis(a, -1, axis)


def _host_K(y):
    """Smallest K so that the optimal pass-2 offset is <= K for every pixel.

    For any pixel p, d2(p) <= min(distW(p), distH(p))^2 and the optimal
    vertical offset satisfies |i-u*| <= sqrt(d2(p)). Empty masks contribute 0
    (the clamp handles them exactly); a mask leaving some pixel with neither a
    same-row nor same-column True falls back to the full K=255 sweep.
    """
    K = 1
    for b in range(B):
        yb = y[b, 0]
        for c in range(C):
            pos = yb == c
            for mask in (pos,):
                if not mask.any():
                    continue
                v = np.minimum(_dist1d(mask, 0), _dist1d(mask, 1))
                vmax = float(v.max())
                if vmax > 1e8:
                    return 255
                K = max(K, int(np.ceil(vmax)))
    return min(K, 255)


def kernel(x, y_):
    global LAST_RESULT
    x = np.ascontiguousarray(np.asarray(x, dtype=np.float32))
    y_ = np.ascontiguousarray(np.asarray(y_, dtype=np.int32))
    assert x.shape == (B, C, H, W) and y_.shape == (B, 1, H, W)

    K = _host_K(y_)
    nc = _build(K)

    in_maps = [{"x": x[b], "y_": y_[b]} for b in range(B)]
    trace = bool(int(os.environ.get("BD_TRACE", "0")))
    res = run_bass_kernel_spmd(nc, in_maps, core_ids=list(range(B)), trace=trace)
    LAST_RESULT = res
    total = sum(r["out"].astype(np.float64).sum() for r in res.results)
    return np.float32(total / (B * C * H * W))
